# revision 54
# baseline (speedup 1.0000x reference)
"""Trainium2 Bass kernel for nn_CapsuleLayer (capsule conv + 3-iter routing).

Reference (per batch image, C=128, H=W=32, K=3, pad=1):
  priors[h,w,t,nc] = sum_c x_pad[c,h+i,w+j] * W[t,c,nc] + b[t,nc]
  o = mean_t priors
  3x: d2 = sum_cch (o - p_t)^2 ; cw = rsqrt(d2 + 1e-4)
      cw = cw / sum_t cw ; o = sum_t cw_t p_t
  out[nc,h,w] = o

Sharding: data-parallel over batch; 8 cores, one image each; weight/bias
replicated; no collectives.

Implementation notes:
- bf16 on-chip (fp32 PSUM accumulation in matmuls); rel err ~8e-3 vs the
  fp32 reference, within the 2e-2 gate.
- priors layout [128pos, grp, tap9, cch16, cap32]: innermost step-1 cap
  runs keep every big DVE op in 2x_1P mode; routing processes GRP=2
  position-chunks per pass to amortize per-op overhead.
- ||o - p||^2 = ||p||^2 - <2o, p> + ||o||^2: per iteration only two
  full-size DVE passes (product o2*p, product p*alpha) plus halving-add
  reductions (2x mode; 1x tensor_reduce avoided for bulk work).
- mean_t priors via 9 extra accumulating matmuls on the idle PE.
- rsqrt = Abs_reciprocal_sqrt on ACT: lives in one table set together
  with square/copy/identity -> no ACT table switching.
- ||o||^2 = sum_t alpha_t <o, p_t> reuses the s-values (no extra pass).
"""

import numpy as np

C = 128
H = W = 32
B = 8
KK = 9
NCAPS = 32
CCH = 16
NC = NCAPS * CCH  # 512
NIT = 3
NPOS = H * W
CHUNK = 128
GRP = 2  # position-chunks per routing pass
NGRP = NPOS // (CHUNK * GRP)
PADW = 34

_cache = {}


def _build(with_bias: bool):
    import concourse.bass as bass
    import concourse.tile as tile
    from concourse import bacc, mybir
    from concourse.masks import make_identity

    f32 = mybir.dt.float32
    bf16 = mybir.dt.bfloat16
    X = mybir.AxisListType.X
    ADD = mybir.AluOpType.add
    MULT = mybir.AluOpType.mult
    AF = mybir.ActivationFunctionType

    nc = bacc.Bacc()
    x_d = nc.dram_tensor("x", [C, H, W], f32, kind="ExternalInput")
    w_d = nc.dram_tensor("w", [KK, C, NC], f32, kind="ExternalInput")
    b_d = nc.dram_tensor("b", [KK, NC], f32, kind="ExternalInput")
    out_d = nc.dram_tensor("out", [NC, NPOS], f32, kind="ExternalOutput")

    with tile.TileContext(nc) as tc:
        with (
            tc.tile_pool(name="singles", bufs=1) as singles,
            tc.tile_pool(name="stage", bufs=1) as stage_pool,
            tc.tile_pool(name="priors", bufs=2) as priors_pool,
            tc.tile_pool(name="big", bufs=3) as big_pool,
            tc.tile_pool(name="half", bufs=2) as half_pool,
            tc.tile_pool(name="o", bufs=3) as o_pool,
            tc.tile_pool(name="small", bufs=3) as small_pool,
            tc.tile_pool(name="pp", bufs=5, space="PSUM") as pp,
            tc.tile_pool(name="mp", bufs=2, space="PSUM") as mp,
            tc.tile_pool(name="tpp", bufs=1, space="PSUM") as tpp,
        ):
            # ---- stage inputs: pad + cast to bf16 via gpsimd cast-DMA ----
            xpad = singles.tile([C, PADW * PADW], bf16)
            xpad_v = xpad[:].rearrange("p (h w) -> p h w", h=PADW)
            # zero only the border so the interior cast-DMA needn't wait on
            # a full-tile memset
            nc.gpsimd.memset(xpad_v[:, 0], 0.0)
            nc.gpsimd.memset(xpad_v[:, PADW - 1], 0.0)
            nc.gpsimd.memset(xpad_v[:, 1 : PADW - 1, 0], 0.0)
            nc.gpsimd.memset(xpad_v[:, 1 : PADW - 1, PADW - 1], 0.0)
            nc.gpsimd.dma_start(out=xpad_v[:, 1 : H + 1, 1 : W + 1], in_=x_d[:])

            # column-shifted padded images: xs[j][c, r*32+w] = xpad[c, r, w+j]
            # -> the (i,j)-tap patch for rows h0.. is the CONTIGUOUS slice
            #    xs[j][:, (h0+i)*32 : (h0+i)*32+128]
            xs = []
            for j in range(3):
                xj = singles.tile([C, PADW * W], bf16, tag=f"xs{j}")
                nc.sync.dma_start(
                    out=xj[:].rearrange("p (r w) -> p r w", r=PADW),
                    in_=xpad_v[:, :, j : j + W],
                )
                xs.append(xj)

            # W: natural-order bf16 load (cast in DMA), then per-tap ACT
            # permute (cap,cch)->(cch,cap) so the matmul rhs is contiguous
            wsb = []
            for t in range(KK):
                wt = singles.tile([C, NC], bf16, tag=f"wsb{t}")
                nc.gpsimd.dma_start(out=wt[:], in_=w_d[t])
                wp_t = singles.tile([C, CCH, NCAPS], bf16, tag=f"wsbp{t}")
                nc.scalar.copy(
                    out=wp_t[:],
                    in_=wt[:].rearrange("p (cap cch) -> p cch cap", cch=CCH),
                )
                wsb.append(wp_t)

            ident = singles.tile([128, 128], f32)
            make_identity(nc, ident[:])

            eps = singles.tile([128, 1], f32)
            nc.gpsimd.memset(eps, 1e-4)

            if with_bias:
                braw = singles.tile([1, KK, NC], bf16)
                nc.gpsimd.dma_start(out=braw[:], in_=b_d[:].unsqueeze(0))
                bsb = singles.tile([1, KK, CCH, NCAPS], bf16)
                nc.scalar.copy(
                    out=bsb[:],
                    in_=braw[:].rearrange("p t (cap cch) -> p t cch cap", cch=CCH),
                )
                ones = singles.tile([1, CHUNK], bf16)
                nc.gpsimd.memset(ones, 1.0)

            for g in range(NGRP):
                # ---- priors + mean via PE ----
                priors = priors_pool.tile([128, GRP, KK, CCH, NCAPS], bf16)
                o2 = o_pool.tile([128, GRP, NC], bf16)
                for cc in range(GRP):
                    ch = GRP * g + cc
                    om = mp.tile([128, NC], f32)  # sum_t priors (fp32)
                    for t in range(KK):
                        i, j = divmod(t, 3)
                        ps = pp.tile([128, NC], f32)
                        lhsT = xs[j][:, 128 * ch + 32 * i : 128 * ch + 32 * i + 128]
                        rhs = wsb[t][:].rearrange("p a b -> p (a b)")
                        if with_bias:
                            nc.tensor.matmul(
                                ps[:], lhsT, rhs, start=True, stop=False
                            )
                            brhs = bsb[:, t].rearrange("p a b -> p (a b)")
                            nc.tensor.matmul(
                                ps[:], ones[:], brhs, start=False, stop=True
                            )
                        else:
                            nc.tensor.matmul(ps[:], lhsT, rhs, start=True, stop=True)
                        nc.tensor.matmul(
                            om[:], lhsT, rhs, start=(t == 0), stop=(t == KK - 1)
                        )
                        if with_bias:
                            nc.tensor.matmul(
                                om[:], ones[:], brhs, start=False, stop=False,
                                skip_group_check=True,
                            )
                        if g == 0 and cc == 0:
                            # DVE is idle during the prologue: help drain the
                            # first chunk's PSUM so priors land sooner
                            nc.vector.tensor_copy(
                                out=priors[:, cc, t],
                                in_=ps[:].rearrange("p (a b) -> p a b", a=CCH),
                            )
                        else:
                            nc.scalar.copy(
                                out=priors[:, cc, t],
                                in_=ps[:].rearrange("p (a b) -> p a b", a=CCH),
                            )
                    # o2 = 2*mean = (2/9) sum_t priors  (bf16)
                    nc.scalar.activation(
                        out=o2[:, cc], in_=om[:], func=AF.Copy, scale=2.0 / KK
                    )

                alpha = None
                ntile = None
                if g > 0:
                    # emit n = sum_cch p^2 before the iteration chain: its
                    # square (ACT) and halvings (DVE) are ready early and
                    # fill the previous group's rsqrt-round-trip stalls.
                    # (For g==0 it stays deferred to shorten the prologue.)
                    tprod = big_pool.tile(
                        [128, GRP, KK, CCH, NCAPS], bf16, tag="big"
                    )
                    nc.scalar.activation(
                        out=tprod[:], in_=priors[:], func=AF.Square
                    )
                    h1 = half_pool.tile([128, GRP, KK, 8, NCAPS], bf16, tag="h1")
                    nc.vector.tensor_add(
                        h1[:], tprod[:, :, :, 0:8], tprod[:, :, :, 8:16]
                    )
                    nc.vector.tensor_add(
                        h1[:, :, :, 0:4], h1[:, :, :, 0:4], h1[:, :, :, 4:8]
                    )
                    nc.vector.tensor_add(
                        h1[:, :, :, 0:2], h1[:, :, :, 0:2], h1[:, :, :, 2:4]
                    )
                    ntile = small_pool.tile([128, GRP, KK, NCAPS], bf16, tag="n")
                    nc.vector.tensor_add(
                        ntile[:], h1[:, :, :, 0], h1[:, :, :, 1]
                    )
                for it in range(NIT):
                    last = it == NIT - 1
                    # s = <o2, p_t>: product + cch halving reduction
                    tprod = big_pool.tile(
                        [128, GRP, KK, CCH, NCAPS], bf16, tag="big"
                    )
                    ob = (
                        o2[:]
                        .rearrange("p c (a b) -> p c a b", a=CCH)
                        .unsqueeze(2)
                        .broadcast_to((128, GRP, KK, CCH, NCAPS))
                    )
                    nc.vector.tensor_mul(tprod[:], priors[:], ob)
                    h1 = half_pool.tile([128, GRP, KK, 8, NCAPS], bf16, tag="h1")
                    nc.vector.tensor_add(
                        h1[:], tprod[:, :, :, 0:8], tprod[:, :, :, 8:16]
                    )
                    nc.vector.tensor_add(
                        h1[:, :, :, 0:4], h1[:, :, :, 0:4], h1[:, :, :, 4:8]
                    )
                    nc.vector.tensor_add(
                        h1[:, :, :, 0:2], h1[:, :, :, 0:2], h1[:, :, :, 2:4]
                    )
                    s = small_pool.tile([128, GRP, KK, NCAPS], bf16, tag="s")
                    nc.vector.tensor_add(s[:], h1[:, :, :, 0], h1[:, :, :, 1])

                    if ntile is None:
                        # n[t,cap] = sum_cch p^2, emitted after the first
                        # product so it doesn't gate the DVE pipeline start
                        tprod = big_pool.tile(
                            [128, GRP, KK, CCH, NCAPS], bf16, tag="big"
                        )
                        nc.scalar.activation(
                            out=tprod[:], in_=priors[:], func=AF.Square
                        )
                        h1 = half_pool.tile(
                            [128, GRP, KK, 8, NCAPS], bf16, tag="h1"
                        )
                        nc.vector.tensor_add(
                            h1[:], tprod[:, :, :, 0:8], tprod[:, :, :, 8:16]
                        )
                        nc.vector.tensor_add(
                            h1[:, :, :, 0:4], h1[:, :, :, 0:4], h1[:, :, :, 4:8]
                        )
                        nc.vector.tensor_add(
                            h1[:, :, :, 0:2], h1[:, :, :, 0:2], h1[:, :, :, 2:4]
                        )
                        ntile = small_pool.tile(
                            [128, GRP, KK, NCAPS], bf16, tag="n"
                        )
                        nc.vector.tensor_add(
                            ntile[:], h1[:, :, :, 0], h1[:, :, :, 1]
                        )

                    # e2 = sum_t alpha_t s_t -> e' = scale*e2 + eps (ACT)
                    e2 = small_pool.tile([128, GRP, NCAPS], f32, tag="e2")
                    if alpha is None:
                        red = s
                        escale = 1.0 / (2 * KK)
                    else:
                        tm = small_pool.tile(
                            [128, GRP, KK, NCAPS], bf16, tag="tm"
                        )
                        nc.vector.tensor_mul(tm[:], alpha[:], s[:])
                        red = tm
                        escale = 0.25
                    nc.vector.tensor_reduce(
                        out=e2[:],
                        in_=red[:].transpose([0, 1, 3, 2]),
                        axis=X,
                        op=ADD,
                    )
                    e2b = small_pool.tile([128, GRP, NCAPS], bf16, tag="e2b")
                    nc.scalar.activation(
                        out=e2b[:], in_=e2[:], func=AF.Identity,
                        bias=eps[:], scale=escale,
                    )

                    # dist = (n - s) + e'  (bf16, 2x)
                    dist = small_pool.tile([128, GRP, KK, NCAPS], bf16, tag="dist")
                    nc.vector.tensor_sub(dist[:], ntile[:], s[:])
                    nc.vector.tensor_add(
                        dist[:],
                        dist[:],
                        e2b[:].unsqueeze(2).broadcast_to((128, GRP, KK, NCAPS)),
                    )
                    # cwu = dist^-0.5 (single-table-set rsqrt on ACT)
                    cwu = small_pool.tile([128, GRP, KK, NCAPS], bf16, tag="cwu")
                    nc.scalar.activation(
                        out=cwu[:], in_=dist[:], func=AF.Abs_reciprocal_sqrt
                    )
                    # alpha = cwu / sum_t cwu (doubled except last iter)
                    cwsum = small_pool.tile([128, GRP, NCAPS], f32, tag="cwsum")
                    nc.vector.tensor_reduce(
                        out=cwsum[:],
                        in_=cwu[:].transpose([0, 1, 3, 2]),
                        axis=X,
                        op=ADD,
                    )
                    rs = small_pool.tile([128, GRP, NCAPS], f32, tag="rs")
                    nc.vector.reciprocal_approx_fast(rs[:], cwsum[:])
                    rsb = small_pool.tile([128, GRP, NCAPS], bf16, tag="rsb")
                    nc.vector.tensor_scalar_mul(
                        rsb[:], rs[:], 1.0 if last else 2.0
                    )
                    alpha = small_pool.tile([128, GRP, KK, NCAPS], bf16, tag="al")
                    nc.vector.tensor_mul(
                        alpha[:],
                        cwu[:],
                        rsb[:].unsqueeze(2).broadcast_to((128, GRP, KK, NCAPS)),
                    )

                    # o' = sum_t alpha_t p_t: product + tap halving
                    if not last:
                        wprod = big_pool.tile(
                            [128, GRP, KK, CCH, NCAPS], bf16, tag="big"
                        )
                        ab = alpha[:].unsqueeze(3).broadcast_to(
                            (128, GRP, KK, CCH, NCAPS)
                        )
                        nc.vector.tensor_mul(wprod[:], priors[:], ab)
                        wp = wprod[:].rearrange("p c t a b -> p c t (a b)")
                        wh = half_pool.tile([128, GRP, 4, NC], bf16, tag="wh")
                        nc.vector.tensor_add(wh[:], wp[:, :, 0:4], wp[:, :, 4:8])
                        nc.vector.tensor_add(
                            wh[:, :, 0:2], wh[:, :, 0:2], wh[:, :, 2:4]
                        )
                        nc.vector.tensor_add(
                            wh[:, :, 0], wh[:, :, 0], wh[:, :, 1]
                        )
                        o2 = o_pool.tile([128, GRP, NC], bf16)
                        nc.vector.tensor_add(o2[:], wh[:, :, 0], wp[:, :, 8])
                        continue

                    # last iteration: per sub-chunk so the output transposes
                    # overlap the other sub-chunk's weighted sum
                    for cc in range(GRP):
                        ch = GRP * g + cc
                        wprod = big_pool.tile(
                            [128, KK, CCH, NCAPS], bf16, tag="big"
                        )
                        ab = alpha[:, cc].unsqueeze(2).broadcast_to(
                            (128, KK, CCH, NCAPS)
                        )
                        nc.vector.tensor_mul(wprod[:], priors[:, cc], ab)
                        wp = wprod[:].rearrange("p t a b -> p t (a b)")
                        wh = half_pool.tile([128, 4, NC], bf16, tag="wh")
                        nc.vector.tensor_add(wh[:], wp[:, 0:4], wp[:, 4:8])
                        nc.vector.tensor_add(
                            wh[:, 0:2], wh[:, 0:2], wh[:, 2:4]
                        )
                        nc.vector.tensor_add(wh[:, 0], wh[:, 0], wh[:, 1])
                        onat = o_pool.tile([128, NC], f32, tag="onat")
                        nc.vector.tensor_add(
                            onat[:].rearrange(
                                "p (cap cch) -> p cch cap", cch=CCH
                            ),
                            wh[:, 0].rearrange(
                                "p (cch cap) -> p cch cap", cch=CCH
                            ),
                            wp[:, 8].rearrange(
                                "p (cch cap) -> p cch cap", cch=CCH
                            ),
                        )
                        ot = small_pool.tile([128, 4, 128], f32, tag="ostage")
                        for blk in range(4):
                            tp = tpp.tile([128, 128], f32)
                            nc.tensor.transpose(
                                tp[:],
                                onat[:, 128 * blk : 128 * (blk + 1)],
                                ident[:],
                            )
                            nc.scalar.copy(out=ot[:, blk], in_=tp[:])
                        nc.sync.dma_start(
                            out=out_d[:, 128 * ch : 128 * (ch + 1)].rearrange(
                                "(blk n) q -> n blk q", blk=4
                            ),
                            in_=ot[:],
                        )
    nc.compile()
    return nc


def _get_nc(with_bias: bool):
    key = ("nc", with_bias)
    if key not in _cache:
        _cache[key] = _build(with_bias)
    return _cache[key]


def kernel(input, weight, bias, _trace=False):
    from concourse.bass_utils import run_bass_kernel_spmd

    input = np.ascontiguousarray(np.asarray(input, dtype=np.float32))
    w = np.ascontiguousarray(
        np.asarray(weight, dtype=np.float32).reshape(KK, C, NC)
    )
    b = np.ascontiguousarray(np.asarray(bias, dtype=np.float32).reshape(KK, NC))
    with_bias = bool(np.any(b))

    nc = _get_nc(with_bias)
    in_maps = [
        {"x": np.ascontiguousarray(input[i]), "w": w, "b": b} for i in range(B)
    ]
    res = run_bass_kernel_spmd(
        nc, in_maps, core_ids=list(range(B)), trace=_trace
    )
    _cache["last_result"] = res
    out = np.stack(
        [r["out"].reshape(NC, H, W) for r in res.results], axis=0
    )
    return out
